# revision 37
# baseline (speedup 1.0000x reference)
"""InfoVAE loss kernel for Trainium2, data-parallel over batch on 8 NeuronCores.

Reference computation (see problem spec):
    recons_loss = mean((recons - x)^2)                    recons/x: [4096, 3, 64, 64]
    mmd  = km(pz,pz) + km(z,z) - 2*km(pz,z)               z/pz:     [4096, 128]
           where km(a,b) = mean_ij exp(-(|a_i-b_j|^2/D)/sigma), sigma = 2*D*z_var
    kld  = mean_n(-0.5 * sum_d(1 + lv - mu^2 - exp(lv)))
    loss = 5*recons_loss + 1.5*(1/N)*kld + 98.5/(N*(N-1))*mmd
    returns (loss, recons_loss, mmd, -kld)

Sharding: each core owns a 512-row block of the batch. Each core receives z and
prior_z ROTATED so its own rows come first (np.roll by -core*512 on the host);
the RBF blocks it computes are its row block vs column chunks of the rotated
full matrix. For the symmetric pairs k(z,z), k(pz,pz) only chunks 0..4 of 8 are
computed (cyclic tournament cover); the host weights them [1,2,2,2,1] so every
unordered block pair is counted exactly once x2 (+diag x1). The cross pair
k(pz,z) needs all 8 chunks. This cuts MMD matmul+exp work by 25% with an
identical SPMD program on every core.

RBF assembly on device: arg_ij = a_i.b_j/32768 - |a_i|^2/65536 - |b_j|^2/65536.
All matmuls run in bf16 (PE 1-pass vs fp32's LOW_HIGH multi-pass). Numerics are
backward-stable: the norms are computed FROM the bf16-rounded points, so the
result is (nearly) the exact statistic of slightly perturbed inputs and the
k(a,a) diagonal still cancels; measured mmd error vs the fp32 reference ~1e-5.
 - a_i.b_j/32768 : bf16 PE matmul, block lhsT pre-scaled by 2^-15 (exact pow2).
 - -|b_j|^2/65536: K=1 accumulating bf16 matmul (ones x negnorm row).
 - -|a_i|^2/65536: per-partition fp32 bias of the ACT Exp instruction.
ACT's fused accum_out gives the per-partition running sums for free.

MSE: DVE sub (bf16 out) + ACT Square with fused accum. (tensor_tensor_reduce
would fuse square+reduce on DVE but hard-faults the exec unit on this HW /
toolchain - NRT_EXEC_UNIT_UNRECOVERABLE; gpsimd elementwise runs at 0.42x
roofline - both rejected.) Exp tiles are grouped into [P, 2, 512] psum groups
of equal host weight to halve ACT instruction+accum-drain overhead. MSE chunk
DMAs are single 1.5 MB transfers (12 KB contiguous per partition).
"""

import numpy as np

N = 4096
D = 128
NCORES = 8
ROWS = N // NCORES            # 512 rows per core
IMG_F = 3 * 64 * 64           # 12288
P = 128
T_ROW = ROWS // P             # 4 row tiles per core
MSE_CHUNK = 3072
MSE_NCH = IMG_F // MSE_CHUNK  # 4
NMSE = T_ROW * MSE_NCH        # 16 accum columns
NSYM = 5                      # symmetric pairs: window of 5 x 512-col chunks
# exp tiles group window chunks of equal host weight: {0,4}@1, {1,2}@2, {3}@2
SYM_GROUPS = ((0, 4), (1, 2), (3,))
NG = len(SYM_GROUPS)          # 3 accum columns per (pair, t)
NW = 2 * T_ROW * NG           # 24 symmetric-pair accum columns
NCHK = N // 512               # 8 column chunks for the cross pair
NCG = NCHK // 2               # cross chunks paired into 4 exp groups
NMMD = NW + T_ROW * NCG       # 40 accum columns total
Z_VAR = 2.0
SIGMA = 2.0 * D * Z_VAR       # 512
INV_2S = 1.0 / (D * SIGMA / 2.0)   # 1/32768 (exact power of two)
INV_S = 1.0 / (D * SIGMA)          # 1/65536

_CACHE = {}


def _build():
    import concourse.bass as bass
    import concourse.tile as tile
    from concourse import bacc, mybir

    f32 = mybir.dt.float32
    bf16 = mybir.dt.bfloat16
    AF = mybir.ActivationFunctionType
    ALU = mybir.AluOpType
    AX = mybir.AxisListType

    nc = bacc.Bacc("TRN2", target_bir_lowering=False, debug=False,
                   num_devices=NCORES)

    r_blk = nc.dram_tensor("r_blk", [ROWS, IMG_F], f32, kind="ExternalInput").ap()
    x_blk = nc.dram_tensor("x_blk", [ROWS, IMG_F], f32, kind="ExternalInput").ap()
    z_full = nc.dram_tensor("z_full", [N, D], f32, kind="ExternalInput").ap()
    pz_full = nc.dram_tensor("pz_full", [N, D], f32, kind="ExternalInput").ap()
    mu_blk = nc.dram_tensor("mu_blk", [ROWS, D], f32, kind="ExternalInput").ap()
    lv_blk = nc.dram_tensor("lv_blk", [ROWS, D], f32, kind="ExternalInput").ap()
    ident = nc.dram_tensor("ident", [P, P], f32, kind="ExternalInput").ap()

    mse_out = nc.dram_tensor("mse_acc", [P, NMSE], f32, kind="ExternalOutput").ap()
    mmd_out = nc.dram_tensor("mmd_acc", [P, NMMD], f32, kind="ExternalOutput").ap()
    kld_out = nc.dram_tensor("kld_acc", [P, 4], f32, kind="ExternalOutput").ap()

    with tile.TileContext(nc) as tc:
        with (
            tc.tile_pool(name="consts", bufs=1) as consts,
            tc.tile_pool(name="nat", bufs=1) as nat,
            tc.tile_pool(name="stream", bufs=5) as stream,
            tc.tile_pool(name="dpool", bufs=2) as dpool,
            tc.tile_pool(name="tstage", bufs=2) as tstage,
            tc.tile_pool(name="scratch", bufs=2) as scratch,
            tc.tile_pool(name="acc", bufs=1) as accp,
            tc.tile_pool(name="ps5", bufs=3, space="PSUM") as ps5,
            tc.tile_pool(name="pstr", bufs=2, space="PSUM") as pstr,
        ):
            # ---- constants / small setup ----
            ident_sb = consts.tile([P, P], f32)
            nc.sync.dma_start(out=ident_sb[:], in_=ident)
            ident16 = consts.tile([P, P], bf16)
            nc.vector.tensor_copy(ident16[:], ident_sb[:])
            ones16 = consts.tile([1, P], bf16)
            nc.vector.memset(ones16[:], 1.0)
            negs16 = consts.tile([P, 1], bf16)      # -1/65536 column (exact pow2)
            nc.vector.memset(negs16[:], -INV_S)

            # accumulators
            mse_cols = accp.tile([P, NMSE], f32)
            mmd_cols = accp.tile([P, NMMD], f32)
            kld_cols = accp.tile([P, 4], f32)
            nc.vector.memset(kld_cols[:, 3:4], 0.0)

            # p-major within each 512-row chunk: partition p holds 4 CONSECUTIVE
            # dram rows per chunk (2 KB descriptors instead of 512 B). This
            # permutes the j-order within chunks only; sums, norms, bias and
            # block-lhsT orderings all stay mutually consistent.
            zv = z_full.rearrange("(c p q) d -> p c q d", p=P, q=4)
            pv = pz_full.rearrange("(c p q) d -> p c q d", p=P, q=4)

            # ---- transpose z/pz to [d, j] bf16 layout via PE (staged loads) ----
            # pz only needs window chunks 0..4 (tiles 0..19); z needs all 32.
            zT16 = consts.tile([P, N], bf16)
            pzT16 = consts.tile([P, N], bf16)
            zb16 = nat.tile([P, T_ROW, D], bf16)   # core's own rows, natural
            pb16 = nat.tile([P, T_ROW, D], bf16)
            for (view, dst, blk16, nchunk) in ((zv, zT16, zb16, 8),
                                               (pv, pzT16, pb16, 5)):
                for base in range(0, nchunk, 2):   # stage 2 chunks (1 MB) at a time
                    cc = min(2, nchunk - base)
                    # SWDGE cast-DMA straight to bf16; same queue as the mse
                    # stream, so staging issues first in program order
                    st16 = tstage.tile([P, 2, 4, D], bf16, tag="tst16")
                    nc.gpsimd.dma_start(out=st16[:, 0:cc, :, :],
                                        in_=view[:, base:base + cc, :, :])
                    if base == 0:
                        # block rows for the ACT bias squares: [p, q, d] with
                        # q = emission tile index t (point row p*4 + t)
                        nc.vector.tensor_copy(blk16[:], st16[:, 0, :, :])
                    for gg in range(cc):          # 4 transposes per psum tile
                        tp = pstr.tile([P, 512], bf16, tag="tr16")
                        for k in range(4):
                            nc.tensor.transpose(tp[:, k * P:(k + 1) * P],
                                                st16[:, gg, k, :], ident16[:])
                        col = (base + gg) * 512
                        nc.vector.tensor_copy(dst[:, col:col + 512], tp[:])

            # ---- column norm rows from the bf16 points: -|b_j|^2/65536, [1, N] ----
            nn_z16 = consts.tile([1, N], bf16)
            nn_pz16 = consts.tile([1, N], bf16)
            for (srcT, dst, nch) in ((zT16, nn_z16, 8), (pzT16, nn_pz16, 5)):
                for c in range(nch):
                    sq = scratch.tile([P, 512], bf16, tag="sqn")
                    nc.vector.tensor_mul(sq[:], srcT[:, c * 512:(c + 1) * 512],
                                         srcT[:, c * 512:(c + 1) * 512])
                    npm = ps5.tile([P, 2, 512], f32, tag="ps")
                    nc.tensor.matmul(npm[0:1, 0, :], lhsT=negs16[:], rhs=sq[:],
                                     start=True, stop=True)
                    nc.vector.tensor_copy(dst[0:1, c * 512:(c + 1) * 512],
                                          npm[0:1, 0, :])

            # ---- block lhsT: first 512 cols of the transposed fulls, x 2^-15 ----
            # bf16(x) * 2^-15 is exact, so this equals bf16(x * 2^-15)
            zbTs16 = consts.tile([P, ROWS], bf16)
            pbTs16 = consts.tile([P, ROWS], bf16)
            nc.vector.tensor_scalar_mul(zbTs16[:], zT16[:, 0:ROWS], INV_2S)
            nc.vector.tensor_scalar_mul(pbTs16[:], pzT16[:, 0:ROWS], INV_2S)

            # ---- row-bias tiles: bias_a[:, t] = -|a_i|^2/65536 from bf16 points ----
            bias_z = consts.tile([P, T_ROW], f32)
            bias_pz = consts.tile([P, T_ROW], f32)
            for (src, dst) in ((zb16, bias_z), (pb16, bias_pz)):
                for t in range(T_ROW):
                    sq2 = scratch.tile([P, D], bf16, tag="sq2")
                    # Square(x/256) = x^2/65536 (scale is an exact pow2)
                    nc.scalar.activation(out=sq2[:], in_=src[:, t, :],
                                         func=AF.Square, scale=1.0 / 256.0,
                                         accum_out=dst[:, t:t + 1])
                nc.vector.tensor_scalar_mul(dst[:], dst[:], -1.0)

            # ---- KLD block terms ----
            # p-major: 4 consecutive rows per partition (2 KB descriptors);
            # the kld sums are row-order invariant
            mu_t = nat.tile([P, T_ROW, D], f32)
            lv_t = nat.tile([P, T_ROW, D], f32)
            nc.sync.dma_start(out=mu_t[:], in_=mu_blk.rearrange("(p q) d -> p q d", p=P))
            nc.sync.dma_start(out=lv_t[:], in_=lv_blk.rearrange("(p q) d -> p q d", p=P))
            ksc = scratch.tile([P, T_ROW, D], f32, tag="ksc")
            nc.vector.tensor_reduce(kld_cols[:, 0:1], lv_t[:], axis=AX.XY,
                                    op=ALU.add)
            nc.scalar.activation(out=ksc[:], in_=mu_t[:], func=AF.Square,
                                 accum_out=kld_cols[:, 1:2])
            ksc2 = scratch.tile([P, T_ROW, D], f32, tag="ksc")
            nc.scalar.activation(out=ksc2[:], in_=lv_t[:], func=AF.Exp,
                                 accum_out=kld_cols[:, 2:3])

            # ---- interleaved main loops: MMD rbf blocks + MSE stream ----
            rv = r_blk.rearrange("(t p) f -> p t f", p=P)
            xv = x_blk.rearrange("(t p) f -> p t f", p=P)

            def emit_mse(k):
                t, c = divmod(k, MSE_NCH)
                # SWDGE cast-during-DMA: fp32 in HBM -> bf16 in SBUF, so the
                # sub runs all-bf16 on DVE (2x rate) and SBUF halves
                rt = stream.tile([P, MSE_CHUNK], bf16, tag="rt")
                xt = stream.tile([P, MSE_CHUNK], bf16, tag="xt")
                if k < 4:
                    # ordering: the mse stream shares the 16 SDMA engines with
                    # the z/pz staging; the first 4 buffer pairs have no data
                    # dep and would flood them from t=0, delaying the MMD
                    # pipeline start. Write one element from the last staged
                    # tensor so these dmas wait for staging completion.
                    nc.gpsimd.tensor_copy(rt[0:1, 0:1], pzT16[0:1, 0:1])
                    nc.gpsimd.tensor_copy(xt[0:1, 0:1], pzT16[0:1, 0:1])
                lo = c * MSE_CHUNK
                nc.gpsimd.dma_start(out=rt[:], in_=rv[:, t, lo:lo + MSE_CHUNK])
                nc.gpsimd.dma_start(out=xt[:], in_=xv[:, t, lo:lo + MSE_CHUNK])
                dt = dpool.tile([P, MSE_CHUNK], bf16, tag="dt")
                nc.vector.tensor_sub(dt[:], rt[:], xt[:])
                sc = dpool.tile([P, MSE_CHUNK], bf16, tag="msq")
                if k % 8 not in (2, 5, 7):
                    nc.scalar.activation(out=sc[:], in_=dt[:], func=AF.Square,
                                         accum_out=mse_cols[:, k:k + 1])
                else:
                    # balance ACT vs DVE: square+reduce on DVE for 6 of 16
                    nc.vector.tensor_mul(sc[:], dt[:], dt[:])
                    nc.vector.tensor_reduce(mse_cols[:, k:k + 1], sc[:],
                                            axis=AX.X, op=ALU.add)

            # one emission = one accum column: a group of 1-2 column chunks
            # matmul'd into a [P, 2, 512] psum tile, one Exp+accum over it
            def emit_group(col, aTs, bT, nn_b, bias_a, t, chunks):
                ps = ps5.tile([P, 2, 512], f32, tag="ps")
                for s, o in enumerate(chunks):
                    nc.tensor.matmul(ps[:, s, :],
                                     lhsT=aTs[:, t * P:(t + 1) * P],
                                     rhs=bT[:, o * 512:(o + 1) * 512],
                                     start=True, stop=False)
                    nc.tensor.matmul(ps[:, s, :], lhsT=ones16[:],
                                     rhs=nn_b[0:1, o * 512:(o + 1) * 512],
                                     start=False, stop=True)
                view = ps[:] if len(chunks) == 2 else ps[:, 0, :]
                sc = scratch.tile([P, 2, 512], bf16, tag="esc")
                scv = sc[:] if len(chunks) == 2 else sc[:, 0, :]
                nc.scalar.activation(out=scv, in_=view, func=AF.Exp,
                                     bias=bias_a[:, t:t + 1], scale=1.0,
                                     accum_out=mmd_cols[:, col:col + 1])

            emits = []
            for pi, (aTs, bT, nn_b, bias_a) in enumerate(
                    ((pbTs16, pzT16, nn_pz16, bias_pz),
                     (zbTs16, zT16, nn_z16, bias_z))):
                for t in range(T_ROW):
                    for g, chunks in enumerate(SYM_GROUPS):
                        col = pi * T_ROW * NG + t * NG + g
                        emits.append((col, aTs, bT, nn_b, bias_a, t, chunks))
            for t in range(T_ROW):
                for g in range(NCG):
                    col = NW + t * NCG + g
                    emits.append((col, pbTs16, zT16, nn_z16, bias_pz, t,
                                  (2 * g, 2 * g + 1)))

            # pace 16 mse chunks across the first ~80% of mmd emissions so the
            # dma stream finishes early and the tail is compute-only
            NEMIT = len(emits)
            PACE = (NEMIT * 7) // 8
            mi = 0
            for k, e in enumerate(emits):
                emit_group(*e)
                while mi * PACE <= k * NMSE and mi < NMSE:
                    emit_mse(mi)
                    mi += 1
            while mi < NMSE:
                emit_mse(mi)
                mi += 1

            # ---- write partials out ----
            nc.sync.dma_start(out=mse_out, in_=mse_cols[:])
            nc.sync.dma_start(out=mmd_out, in_=mmd_cols[:])
            nc.sync.dma_start(out=kld_out, in_=kld_cols[:])

    nc.compile()
    return nc


def get_nc():
    if "nc" not in _CACHE:
        _CACHE["nc"] = _build()
    return _CACHE["nc"]


def make_in_maps(recons, x, z, mu, log_var, prior_z):
    r2 = np.ascontiguousarray(recons, dtype=np.float32).reshape(N, IMG_F)
    x2 = np.ascontiguousarray(x, dtype=np.float32).reshape(N, IMG_F)
    z = np.ascontiguousarray(z, dtype=np.float32)
    pz = np.ascontiguousarray(prior_z, dtype=np.float32)
    mu = np.ascontiguousarray(mu, dtype=np.float32)
    lv = np.ascontiguousarray(log_var, dtype=np.float32)
    ident = np.eye(P, dtype=np.float32)
    maps = []
    for c in range(NCORES):
        s = slice(c * ROWS, (c + 1) * ROWS)
        # rotate so this core's rows come first; its symmetric window is then
        # always chunks 0..4 and the cross pair sees all of z (reordered)
        zr = np.roll(z, -c * ROWS, axis=0)
        pzr = np.roll(pz, -c * ROWS, axis=0)
        maps.append({
            "r_blk": r2[s], "x_blk": x2[s],
            "z_full": zr, "pz_full": pzr,
            "mu_blk": mu[s], "lv_blk": lv[s],
            "ident": ident,
        })
    return maps


# host-side weights for the symmetric-window accumulator groups {0,4},{1,2},{3}
_SYM_W = np.array([1.0, 2.0, 2.0])


def combine(results):
    mse_sum = 0.0
    s_pp = s_zz = s_pz = 0.0
    kld_total = 0.0
    for res in results:
        mse_sum += np.float64(res["mse_acc"]).sum()
        m = np.float64(res["mmd_acc"])
        # symmetric windows: cols [pair(2), t(4), g(3)] with weights over g
        sym = m[:, 0:NW].reshape(P, 2, T_ROW, NG)
        w = (sym * _SYM_W).sum(axis=(0, 2, 3))
        s_pp += w[0]
        s_zz += w[1]
        s_pz += m[:, NW:].sum()
        k = np.float64(res["kld_acc"])
        kld_total += ROWS * D + k[:, 0].sum() - k[:, 1].sum() - k[:, 2].sum()

    recons_loss = mse_sum / (N * IMG_F)
    mmd = (s_pp + s_zz - 2.0 * s_pz) / (float(N) * float(N))
    kld = -0.5 * kld_total / N
    beta, alpha, reg_w = 5.0, -0.5, 100.0
    loss = (beta * recons_loss
            + (1.0 - alpha) * (1.0 / N) * kld
            + (alpha + reg_w - 1.0) / (float(N) * (N - 1)) * mmd)
    return (np.float32(loss), np.float32(recons_loss),
            np.float32(mmd), np.float32(-kld))


def run(recons, x, z, mu, log_var, prior_z, trace=False):
    from concourse.bass_utils import run_bass_kernel_spmd
    nc = get_nc()
    in_maps = make_in_maps(recons, x, z, mu, log_var, prior_z)
    res = run_bass_kernel_spmd(nc, in_maps, list(range(NCORES)), trace=trace)
    return res


def kernel(recons, x, z, mu, log_var, prior_z):
    res = run(recons, x, z, mu, log_var, prior_z)
    return combine(res.results)


# revision 38
# speedup vs baseline: 1.1207x; 1.1207x over previous
"""InfoVAE loss kernel for Trainium2, data-parallel over batch on 8 NeuronCores.

Reference computation (see problem spec):
    recons_loss = mean((recons - x)^2)                    recons/x: [4096, 3, 64, 64]
    mmd  = km(pz,pz) + km(z,z) - 2*km(pz,z)               z/pz:     [4096, 128]
           where km(a,b) = mean_ij exp(-(|a_i-b_j|^2/D)/sigma), sigma = 2*D*z_var
    kld  = mean_n(-0.5 * sum_d(1 + lv - mu^2 - exp(lv)))
    loss = 5*recons_loss + 1.5*(1/N)*kld + 98.5/(N*(N-1))*mmd
    returns (loss, recons_loss, mmd, -kld)

Sharding: each core owns a 512-row block of the batch. Each core receives z and
prior_z ROTATED so its own rows come first (np.roll by -core*512 on the host);
the RBF blocks it computes are its row block vs column chunks of the rotated
full matrix. For the symmetric pairs k(z,z), k(pz,pz) only chunks 0..4 of 8 are
computed (cyclic tournament cover); the host weights them [1,2,2,2,1] so every
unordered block pair is counted exactly once x2 (+diag x1). The cross pair
k(pz,z) needs all 8 chunks. This cuts MMD matmul+exp work by 25% with an
identical SPMD program on every core.

RBF assembly on device: arg_ij = a_i.b_j/32768 - |a_i|^2/65536 - |b_j|^2/65536.
All matmuls run in bf16 (PE 1-pass vs fp32's LOW_HIGH multi-pass). Numerics are
backward-stable: the norms are computed FROM the bf16-rounded points, so the
result is (nearly) the exact statistic of slightly perturbed inputs and the
k(a,a) diagonal still cancels; measured mmd error vs the fp32 reference ~1e-5.
 - a_i.b_j/32768 : bf16 PE matmul, block lhsT pre-scaled by 2^-15 (exact pow2).
 - -|b_j|^2/65536: K=1 accumulating bf16 matmul (ones x negnorm row).
 - -|a_i|^2/65536: per-partition fp32 bias of the ACT Exp instruction.
ACT's fused accum_out gives the per-partition running sums for free.

MSE: DVE sub (bf16 out) + ACT Square with fused accum. (tensor_tensor_reduce
would fuse square+reduce on DVE but hard-faults the exec unit on this HW /
toolchain - NRT_EXEC_UNIT_UNRECOVERABLE; gpsimd elementwise runs at 0.42x
roofline - both rejected.) Exp tiles are grouped into [P, 2, 512] psum groups
of equal host weight to halve ACT instruction+accum-drain overhead. MSE chunk
DMAs are single 1.5 MB transfers (12 KB contiguous per partition).
"""

import numpy as np

N = 4096
D = 128
NCORES = 8
ROWS = N // NCORES            # 512 rows per core
IMG_F = 3 * 64 * 64           # 12288
P = 128
T_ROW = ROWS // P             # 4 row tiles per core
MSE_CHUNK = 3072
MSE_NCH = IMG_F // MSE_CHUNK  # 4
NMSE = T_ROW * MSE_NCH        # 16 accum columns
NSYM = 5                      # symmetric pairs: window of 5 x 512-col chunks
# exp tiles group window chunks of equal host weight: {0,4}@1, {1,2}@2, {3}@2
SYM_GROUPS = ((0, 4), (1, 2), (3,))
NG = len(SYM_GROUPS)          # 3 accum columns per (pair, t)
NW = 2 * T_ROW * NG           # 24 symmetric-pair accum columns
NCHK = N // 512               # 8 column chunks for the cross pair
NCG = NCHK // 2               # cross chunks paired into 4 exp groups
NMMD = NW + T_ROW * NCG       # 40 accum columns total
Z_VAR = 2.0
SIGMA = 2.0 * D * Z_VAR       # 512
INV_2S = 1.0 / (D * SIGMA / 2.0)   # 1/32768 (exact power of two)
INV_S = 1.0 / (D * SIGMA)          # 1/65536

_CACHE = {}


def _build():
    import concourse.bass as bass
    import concourse.tile as tile
    from concourse import bacc, mybir

    f32 = mybir.dt.float32
    bf16 = mybir.dt.bfloat16
    AF = mybir.ActivationFunctionType
    ALU = mybir.AluOpType
    AX = mybir.AxisListType

    nc = bacc.Bacc("TRN2", target_bir_lowering=False, debug=False,
                   num_devices=NCORES)

    r_blk = nc.dram_tensor("r_blk", [ROWS, IMG_F], f32, kind="ExternalInput").ap()
    x_blk = nc.dram_tensor("x_blk", [ROWS, IMG_F], f32, kind="ExternalInput").ap()
    z_full = nc.dram_tensor("z_full", [N, D], f32, kind="ExternalInput").ap()
    pz_full = nc.dram_tensor("pz_full", [N, D], f32, kind="ExternalInput").ap()
    mu_blk = nc.dram_tensor("mu_blk", [ROWS, D], f32, kind="ExternalInput").ap()
    lv_blk = nc.dram_tensor("lv_blk", [ROWS, D], f32, kind="ExternalInput").ap()
    ident = nc.dram_tensor("ident", [P, P], f32, kind="ExternalInput").ap()

    mse_out = nc.dram_tensor("mse_acc", [P, NMSE], f32, kind="ExternalOutput").ap()
    mmd_out = nc.dram_tensor("mmd_acc", [P, NMMD], f32, kind="ExternalOutput").ap()
    kld_out = nc.dram_tensor("kld_acc", [P, 4], f32, kind="ExternalOutput").ap()

    with tile.TileContext(nc) as tc:
        with (
            tc.tile_pool(name="consts", bufs=1) as consts,
            tc.tile_pool(name="nat", bufs=1) as nat,
            tc.tile_pool(name="stream", bufs=5) as stream,
            tc.tile_pool(name="dpool", bufs=2) as dpool,
            tc.tile_pool(name="tstage", bufs=2) as tstage,
            tc.tile_pool(name="scratch", bufs=2) as scratch,
            tc.tile_pool(name="acc", bufs=1) as accp,
            tc.tile_pool(name="ps5", bufs=3, space="PSUM") as ps5,
            tc.tile_pool(name="pstr", bufs=2, space="PSUM") as pstr,
        ):
            # ---- constants / small setup ----
            ident_sb = consts.tile([P, P], f32)
            nc.sync.dma_start(out=ident_sb[:], in_=ident)
            ident16 = consts.tile([P, P], bf16)
            nc.vector.tensor_copy(ident16[:], ident_sb[:])
            ones16 = consts.tile([1, P], bf16)
            nc.vector.memset(ones16[:], 1.0)
            negs16 = consts.tile([P, 1], bf16)      # -1/65536 column (exact pow2)
            nc.vector.memset(negs16[:], -INV_S)

            # accumulators
            mse_cols = accp.tile([P, NMSE], f32)
            mmd_cols = accp.tile([P, NMMD], f32)
            kld_cols = accp.tile([P, 4], f32)
            nc.vector.memset(kld_cols[:, 3:4], 0.0)

            # p-major within each 512-row chunk: partition p holds 4 CONSECUTIVE
            # dram rows per chunk (2 KB descriptors instead of 512 B). This
            # permutes the j-order within chunks only; sums, norms, bias and
            # block-lhsT orderings all stay mutually consistent.
            zv = z_full.rearrange("(c p q) d -> p c q d", p=P, q=4)
            pv = pz_full.rearrange("(c p q) d -> p c q d", p=P, q=4)

            # ---- transpose z/pz to [d, j] bf16 layout via PE (staged loads) ----
            # pz only needs window chunks 0..4 (tiles 0..19); z needs all 32.
            zT16 = consts.tile([P, N], bf16)
            pzT16 = consts.tile([P, N], bf16)
            zb16 = nat.tile([P, T_ROW, D], bf16)   # core's own rows, natural
            pb16 = nat.tile([P, T_ROW, D], bf16)
            for (view, dst, blk16, nchunk) in ((zv, zT16, zb16, 8),
                                               (pv, pzT16, pb16, 5)):
                for base in range(0, nchunk, 2):   # stage 2 chunks (1 MB) at a time
                    cc = min(2, nchunk - base)
                    # SWDGE cast-DMA straight to bf16; same queue as the mse
                    # stream, so staging issues first in program order
                    st16 = tstage.tile([P, 2, 4, D], bf16, tag="tst16")
                    nc.gpsimd.dma_start(out=st16[:, 0:cc, :, :],
                                        in_=view[:, base:base + cc, :, :])
                    if base == 0:
                        # block rows for the ACT bias squares: [p, q, d] with
                        # q = emission tile index t (point row p*4 + t)
                        nc.vector.tensor_copy(blk16[:], st16[:, 0, :, :])
                    for gg in range(cc):          # 4 transposes per psum tile
                        tp = pstr.tile([P, 512], bf16, tag="tr16")
                        for k in range(4):
                            nc.tensor.transpose(tp[:, k * P:(k + 1) * P],
                                                st16[:, gg, k, :], ident16[:])
                        col = (base + gg) * 512
                        nc.vector.tensor_copy(dst[:, col:col + 512], tp[:])

            # ---- column norm rows from the bf16 points: -|b_j|^2/65536, [1, N] ----
            nn_z16 = consts.tile([1, N], bf16)
            nn_pz16 = consts.tile([1, N], bf16)
            for (srcT, dst, nch) in ((zT16, nn_z16, 8), (pzT16, nn_pz16, 5)):
                for c in range(nch):
                    sq = scratch.tile([P, 512], bf16, tag="sqn")
                    nc.vector.tensor_mul(sq[:], srcT[:, c * 512:(c + 1) * 512],
                                         srcT[:, c * 512:(c + 1) * 512])
                    npm = ps5.tile([P, 2, 512], f32, tag="ps")
                    nc.tensor.matmul(npm[0:1, 0, :], lhsT=negs16[:], rhs=sq[:],
                                     start=True, stop=True)
                    nc.vector.tensor_copy(dst[0:1, c * 512:(c + 1) * 512],
                                          npm[0:1, 0, :])

            # ---- block lhsT: first 512 cols of the transposed fulls, x 2^-15 ----
            # bf16(x) * 2^-15 is exact, so this equals bf16(x * 2^-15)
            zbTs16 = consts.tile([P, ROWS], bf16)
            pbTs16 = consts.tile([P, ROWS], bf16)
            nc.vector.tensor_scalar_mul(zbTs16[:], zT16[:, 0:ROWS], INV_2S)
            nc.vector.tensor_scalar_mul(pbTs16[:], pzT16[:, 0:ROWS], INV_2S)

            # ---- row-bias tiles: bias_a[:, t] = -|a_i|^2/65536 from bf16 points ----
            bias_z = consts.tile([P, T_ROW], f32)
            bias_pz = consts.tile([P, T_ROW], f32)
            for (src, dst) in ((zb16, bias_z), (pb16, bias_pz)):
                for t in range(T_ROW):
                    sq2 = scratch.tile([P, D], bf16, tag="sq2")
                    # Square(x/256) = x^2/65536 (scale is an exact pow2)
                    nc.scalar.activation(out=sq2[:], in_=src[:, t, :],
                                         func=AF.Square, scale=1.0 / 256.0,
                                         accum_out=dst[:, t:t + 1])
                nc.vector.tensor_scalar_mul(dst[:], dst[:], -1.0)

            # ---- KLD block terms ----
            # p-major: 4 consecutive rows per partition (2 KB descriptors);
            # the kld sums are row-order invariant
            mu_t = nat.tile([P, T_ROW, D], f32)
            lv_t = nat.tile([P, T_ROW, D], f32)
            nc.sync.dma_start(out=mu_t[:], in_=mu_blk.rearrange("(p q) d -> p q d", p=P))
            nc.sync.dma_start(out=lv_t[:], in_=lv_blk.rearrange("(p q) d -> p q d", p=P))
            ksc = scratch.tile([P, T_ROW, D], f32, tag="ksc")
            nc.vector.tensor_reduce(kld_cols[:, 0:1], lv_t[:], axis=AX.XY,
                                    op=ALU.add)
            nc.scalar.activation(out=ksc[:], in_=mu_t[:], func=AF.Square,
                                 accum_out=kld_cols[:, 1:2])
            ksc2 = scratch.tile([P, T_ROW, D], f32, tag="ksc")
            nc.scalar.activation(out=ksc2[:], in_=lv_t[:], func=AF.Exp,
                                 accum_out=kld_cols[:, 2:3])

            # ---- interleaved main loops: MMD rbf blocks + MSE stream ----
            rv = r_blk.rearrange("(t p) f -> p t f", p=P)
            xv = x_blk.rearrange("(t p) f -> p t f", p=P)

            def emit_mse(k):
                t, c = divmod(k, MSE_NCH)
                # SWDGE cast-during-DMA: fp32 in HBM -> bf16 in SBUF, so the
                # sub runs all-bf16 on DVE (2x rate) and SBUF halves
                rt = stream.tile([P, MSE_CHUNK], bf16, tag="rt")
                xt = stream.tile([P, MSE_CHUNK], bf16, tag="xt")
                if k < 5:
                    # ordering: the mse stream shares the 16 SDMA engines with
                    # the z/pz staging; the first 4 buffer pairs have no data
                    # dep and would flood them from t=0, delaying the MMD
                    # pipeline start. Write one element from the last staged
                    # tensor so these dmas wait for staging completion.
                    nc.gpsimd.tensor_copy(rt[0:1, 0:1], pzT16[0:1, 0:1])
                    nc.gpsimd.tensor_copy(xt[0:1, 0:1], pzT16[0:1, 0:1])
                lo = c * MSE_CHUNK
                nc.gpsimd.dma_start(out=rt[:], in_=rv[:, t, lo:lo + MSE_CHUNK])
                nc.gpsimd.dma_start(out=xt[:], in_=xv[:, t, lo:lo + MSE_CHUNK])
                dt = dpool.tile([P, MSE_CHUNK], bf16, tag="dt")
                nc.vector.tensor_sub(dt[:], rt[:], xt[:])
                sc = dpool.tile([P, MSE_CHUNK], bf16, tag="msq")
                if k % 8 not in (2, 5, 7):
                    nc.scalar.activation(out=sc[:], in_=dt[:], func=AF.Square,
                                         accum_out=mse_cols[:, k:k + 1])
                else:
                    # balance ACT vs DVE: square+reduce on DVE for 6 of 16
                    nc.vector.tensor_mul(sc[:], dt[:], dt[:])
                    nc.vector.tensor_reduce(mse_cols[:, k:k + 1], sc[:],
                                            axis=AX.X, op=ALU.add)

            # one emission = one accum column: a group of 1-2 column chunks
            # matmul'd into a [P, 2, 512] psum tile, one Exp+accum over it
            def emit_group(col, aTs, bT, nn_b, bias_a, t, chunks):
                ps = ps5.tile([P, 2, 512], f32, tag="ps")
                for s, o in enumerate(chunks):
                    nc.tensor.matmul(ps[:, s, :],
                                     lhsT=aTs[:, t * P:(t + 1) * P],
                                     rhs=bT[:, o * 512:(o + 1) * 512],
                                     start=True, stop=False)
                    nc.tensor.matmul(ps[:, s, :], lhsT=ones16[:],
                                     rhs=nn_b[0:1, o * 512:(o + 1) * 512],
                                     start=False, stop=True)
                view = ps[:] if len(chunks) == 2 else ps[:, 0, :]
                sc = scratch.tile([P, 2, 512], bf16, tag="esc")
                scv = sc[:] if len(chunks) == 2 else sc[:, 0, :]
                nc.scalar.activation(out=scv, in_=view, func=AF.Exp,
                                     bias=bias_a[:, t:t + 1], scale=1.0,
                                     accum_out=mmd_cols[:, col:col + 1])

            emits = []
            for pi, (aTs, bT, nn_b, bias_a) in enumerate(
                    ((pbTs16, pzT16, nn_pz16, bias_pz),
                     (zbTs16, zT16, nn_z16, bias_z))):
                for t in range(T_ROW):
                    for g, chunks in enumerate(SYM_GROUPS):
                        col = pi * T_ROW * NG + t * NG + g
                        emits.append((col, aTs, bT, nn_b, bias_a, t, chunks))
            for t in range(T_ROW):
                for g in range(NCG):
                    col = NW + t * NCG + g
                    emits.append((col, pbTs16, zT16, nn_z16, bias_pz, t,
                                  (2 * g, 2 * g + 1)))

            # pace 16 mse chunks across the first ~80% of mmd emissions so the
            # dma stream finishes early and the tail is compute-only
            NEMIT = len(emits)
            PACE = (NEMIT * 7) // 8
            mi = 0
            for k, e in enumerate(emits):
                emit_group(*e)
                while mi * PACE <= k * NMSE and mi < NMSE:
                    emit_mse(mi)
                    mi += 1
            while mi < NMSE:
                emit_mse(mi)
                mi += 1

            # ---- write partials out ----
            nc.sync.dma_start(out=mse_out, in_=mse_cols[:])
            nc.sync.dma_start(out=mmd_out, in_=mmd_cols[:])
            nc.sync.dma_start(out=kld_out, in_=kld_cols[:])

    nc.compile()
    return nc


def get_nc():
    if "nc" not in _CACHE:
        _CACHE["nc"] = _build()
    return _CACHE["nc"]


def make_in_maps(recons, x, z, mu, log_var, prior_z):
    r2 = np.ascontiguousarray(recons, dtype=np.float32).reshape(N, IMG_F)
    x2 = np.ascontiguousarray(x, dtype=np.float32).reshape(N, IMG_F)
    z = np.ascontiguousarray(z, dtype=np.float32)
    pz = np.ascontiguousarray(prior_z, dtype=np.float32)
    mu = np.ascontiguousarray(mu, dtype=np.float32)
    lv = np.ascontiguousarray(log_var, dtype=np.float32)
    ident = np.eye(P, dtype=np.float32)
    maps = []
    for c in range(NCORES):
        s = slice(c * ROWS, (c + 1) * ROWS)
        # rotate so this core's rows come first; its symmetric window is then
        # always chunks 0..4 and the cross pair sees all of z (reordered)
        zr = np.roll(z, -c * ROWS, axis=0)
        pzr = np.roll(pz, -c * ROWS, axis=0)
        maps.append({
            "r_blk": r2[s], "x_blk": x2[s],
            "z_full": zr, "pz_full": pzr,
            "mu_blk": mu[s], "lv_blk": lv[s],
            "ident": ident,
        })
    return maps


# host-side weights for the symmetric-window accumulator groups {0,4},{1,2},{3}
_SYM_W = np.array([1.0, 2.0, 2.0])


def combine(results):
    mse_sum = 0.0
    s_pp = s_zz = s_pz = 0.0
    kld_total = 0.0
    for res in results:
        mse_sum += np.float64(res["mse_acc"]).sum()
        m = np.float64(res["mmd_acc"])
        # symmetric windows: cols [pair(2), t(4), g(3)] with weights over g
        sym = m[:, 0:NW].reshape(P, 2, T_ROW, NG)
        w = (sym * _SYM_W).sum(axis=(0, 2, 3))
        s_pp += w[0]
        s_zz += w[1]
        s_pz += m[:, NW:].sum()
        k = np.float64(res["kld_acc"])
        kld_total += ROWS * D + k[:, 0].sum() - k[:, 1].sum() - k[:, 2].sum()

    recons_loss = mse_sum / (N * IMG_F)
    mmd = (s_pp + s_zz - 2.0 * s_pz) / (float(N) * float(N))
    kld = -0.5 * kld_total / N
    beta, alpha, reg_w = 5.0, -0.5, 100.0
    loss = (beta * recons_loss
            + (1.0 - alpha) * (1.0 / N) * kld
            + (alpha + reg_w - 1.0) / (float(N) * (N - 1)) * mmd)
    return (np.float32(loss), np.float32(recons_loss),
            np.float32(mmd), np.float32(-kld))


def run(recons, x, z, mu, log_var, prior_z, trace=False):
    from concourse.bass_utils import run_bass_kernel_spmd
    nc = get_nc()
    in_maps = make_in_maps(recons, x, z, mu, log_var, prior_z)
    res = run_bass_kernel_spmd(nc, in_maps, list(range(NCORES)), trace=trace)
    return res


def kernel(recons, x, z, mu, log_var, prior_z):
    res = run(recons, x, z, mu, log_var, prior_z)
    return combine(res.results)
